# revision 33
# baseline (speedup 1.0000x reference)
"""Masked per-sample MSE loss (duration-predictor loss) on 8 Trainium2 cores.

Math (per the reference):
    mask[i, j]  = j < token_lengths[i]
    diff        = where(mask, pred - log(alignment), 0.0)
    out         = mean_i( sum_j diff[i,j]^2 / token_lengths[i] )

Sharding: data parallel over the batch dim, with length-sorted row
assignment. Rows are sorted by token_length; sorted rank r goes to row-tile
t = r // 1024, core c = r % 8, partition p = (r % 1024) // 8. Every core's
row-tile t then spans the same global length range, so one SPMD module
(shapes fixed from the global per-tile max lengths W[t]) fits all cores, and
tile t only needs its first W[t] columns DMA'd — ~62% of the full input
traffic for uniform lengths. Columns are processed in "bands"
[W[b-1], W[b]) covering tiles b..3; sorted order guarantees every tile
above the diagonal is fully valid inside its band, so masking only runs on
diagonal slices.

Per band: chunked DMA + Ln(align) in place + d = pred - la (chunks fill a
static per-band d region), then per tile one Square-with-row-sum-accum,
split between ACT (activation Square) and DVE (scalar_tensor_tensor
d*d with accum) to balance engine load. The final band (tile 3 alone) is
chunked with shrinking widths: per chunk DVE applies the mask
(iota < len) * d and ACT squares it, so the dependency chain after the very
last DMA byte is short. Per-row divide by length and the global mean run on
the host in float64.

Written in raw Bass (explicit semaphores): the walrus build in this
environment rejects compute instructions carrying more than one sync-wait,
so waits are issued as standalone wait_ge instructions.
"""

from contextlib import ExitStack

import numpy as np

import concourse.bass as bass
from concourse import mybir
from concourse.bass_utils import run_bass_kernel_spmd

B, T = 4096, 2048
N_CORES = 8
RPC = B // N_CORES    # rows per core = 512
P = 128               # SBUF partitions
N_TILES = RPC // P    # row-tiles per core = 4
GROUP = P * N_CORES   # sorted ranks per row-tile = 1024

_CACHE: dict = {}

F32 = mybir.dt.float32


def _tail_chunks(width):
    """Shrinking chunks for the final band so the last chain is short."""
    if width <= 128:
        return [width]
    chunks = []
    rem = width
    while rem > 768:
        take = min(1024, rem - 512)
        chunks.append(take)
        rem -= take
    while rem > 96:
        take = max(64, rem // 2)
        chunks.append(take)
        rem -= take
    chunks.append(rem)
    return chunks


def _split_even(width, pieces):
    base = width // pieces
    out = [base] * pieces
    out[0] += width - base * pieces
    return [w for w in out if w > 0]


def _build_plan(W):
    """bands: list of dicts. Each band covers cols [o, o+w) of tiles b..3.

    chunks: list of (o, w) DMA/Ln/d granules.
    Bands 0..2 get one whole-band square per active tile, split across
    ACT/DVE. The last band gets per-chunk masked squares on ACT.
    """
    bands = []
    prev = 0
    for b in range(N_TILES):
        hi = W[b]
        if hi <= prev:
            continue
        width = hi - prev
        last = b == N_TILES - 1
        if last:
            widths = _tail_chunks(width)
        elif b == 0:
            widths = _split_even(width, 3)  # early pipeline start
        else:
            widths = _split_even(width, max(1, width * (N_TILES - b) // 2048))
        chunks = []
        o = prev
        for w in widths:
            chunks.append((o, w))
            o += w
        bands.append({
            "b": b, "o": prev, "w": width,
            "tiles": list(range(b, N_TILES)),
            "chunks": chunks, "last": last,
        })
        prev = hi

    # rs columns + engine assignment for squares
    col = 0
    dve_load = 0.0
    act_load = 0.0
    for band in bands:
        band["rs"] = {}
        if band["last"]:
            # one rs column per chunk, squares on ACT (chain ping-pong)
            for ci in range(len(band["chunks"])):
                band["rs"][ci] = col
                col += 1
        else:
            band["sq_engine"] = {}
            for t in band["tiles"]:
                band["rs"][t] = col
                col += 1
                # the diagonal must route via dm; balance streaming load
                if t == band["b"]:
                    band["sq_engine"][t] = "act"
                    act_load += band["w"]
                elif act_load <= dve_load * 2.5:
                    band["sq_engine"][t] = "act"
                    act_load += band["w"]
                else:
                    band["sq_engine"][t] = "dve"
                    dve_load += band["w"]
    return bands, col


def _build_module(W):
    bands, n_rs = _build_plan(W)
    # flat list of (band_idx, chunk_idx) in processing order
    flat = [(bi, ci) for bi, band in enumerate(bands)
            for ci in range(len(band["chunks"]))]
    nch = len(flat)
    chunk_id = {k: i for i, k in enumerate(flat)}

    # static d region layout: per band, n_tiles_active * width per partition
    d_off = []
    off = 0
    for band in bands:
        d_off.append(off)
        off += len(band["tiles"]) * band["w"]
    d_total = off
    # static dm region layout: per band, the diagonal width
    dm_off = []
    off = 0
    for band in bands:
        dm_off.append(off)
        off += band["w"]
    dm_total = off

    nc = bass.Bass("TRN2")

    pred_d = nc.dram_tensor("pred", [RPC, T], F32, kind="ExternalInput")
    align_d = nc.dram_tensor("align", [RPC, T], F32, kind="ExternalInput")
    lens_d = nc.dram_tensor("lens", [P, N_TILES], F32, kind="ExternalInput")
    out_d = nc.dram_tensor("rowsums", [P, n_rs], F32, kind="ExternalOutput")

    with ExitStack() as ctx:
        pred_sb = ctx.enter_context(nc.sbuf_tensor("pred_sb", [P, N_TILES, T], F32))
        align_sb = ctx.enter_context(nc.sbuf_tensor("align_sb", [P, N_TILES, T], F32))
        # Ln runs in place: la overwrites align
        d_sb = ctx.enter_context(nc.sbuf_tensor("d_sb", [P, d_total], F32))
        dm_sb = ctx.enter_context(nc.sbuf_tensor("dm_sb", [P, dm_total], F32))
        sq_sb = ctx.enter_context(nc.sbuf_tensor("sq_sb", [P, 2, 2048], F32))
        iota_f = ctx.enter_context(nc.sbuf_tensor("iota_f", [P, T], F32))
        lens_sb = ctx.enter_context(nc.sbuf_tensor("lens_sb", [P, N_TILES], F32))
        rs_sb = ctx.enter_context(nc.sbuf_tensor("rs_sb", [P, n_rs], F32))
        s_pred = [ctx.enter_context(nc.semaphore(f"s_pred{i}")) for i in range(nch)]
        s_align = [ctx.enter_context(nc.semaphore(f"s_align{i}")) for i in range(nch)]
        s_la = [ctx.enter_context(nc.semaphore(f"s_la{i}")) for i in range(nch)]
        s_lens = ctx.enter_context(nc.semaphore("s_lens"))
        s_out = ctx.enter_context(nc.semaphore("s_out"))
        s_iota = ctx.enter_context(nc.semaphore("s_iota"))
        s_d = ctx.enter_context(nc.semaphore("s_d"))
        s_dm = ctx.enter_context(nc.semaphore("s_dm"))
        s_sqa = ctx.enter_context(nc.semaphore("s_sqa"))
        s_sqv = ctx.enter_context(nc.semaphore("s_sqv"))
        block = ctx.enter_context(nc.Block())

        def dram_chunk(dram, bi, ci):
            band = bands[bi]
            t0 = band["tiles"][0]
            n = len(band["tiles"])
            o, w = band["chunks"][ci]
            ap = dram[t0 * P:(t0 + n) * P, o:o + w]
            return ap.rearrange("(n p) w -> p n w", p=P)

        def sbuf_chunk(sb, bi, ci):
            band = bands[bi]
            t0 = band["tiles"][0]
            n = len(band["tiles"])
            o, w = band["chunks"][ci]
            if o == 0 and w == T:
                # full rows: keep the AP contiguous for the DMA engines
                return sb[:, t0:t0 + n, :].rearrange("p n w -> p (n w)")
            return sb[:, t0:t0 + n, o:o + w]

        def d_region(bi):
            band = bands[bi]
            n = len(band["tiles"])
            return d_sb[:, d_off[bi]:d_off[bi] + n * band["w"]].rearrange(
                "p (n w) -> p n w", n=n)

        def d_chunk(bi, ci):
            band = bands[bi]
            o, w = band["chunks"][ci]
            rel = o - band["o"]
            return d_region(bi)[:, :, rel:rel + w]

        def dm_chunk(bi, ci):
            band = bands[bi]
            o, w = band["chunks"][ci]
            rel = o - band["o"]
            return dm_sb[:, dm_off[bi] + rel:dm_off[bi] + rel + w]

        # global d/dm op index after each chunk (emission order = flat order)
        d_idx = {k: i + 1 for i, k in enumerate(flat)}
        band_d_done = [d_idx[(bi, len(band["chunks"]) - 1)]
                       for bi, band in enumerate(bands)]
        band_dm_done = band_d_done  # one dm per chunk, same order

        n_sqa_total = 0
        n_sqv_total = 0
        for band in bands:
            if band["last"]:
                n_sqa_total += len(band["chunks"])
            else:
                for t in band["tiles"]:
                    if band["sq_engine"][t] == "act":
                        n_sqa_total += 1
                    else:
                        n_sqv_total += 1

        # hoist the aligns (and Lns) of the tail band's chunks: their Ln is
        # long done when their pred lands, so the final chain is short
        last_bi = len(bands) - 1
        hoist = ([(last_bi, ci) for ci in range(len(bands[last_bi]["chunks"]))]
                 if len(bands) > 1 and bands[last_bi]["last"] else [])
        hoist_set = set(hoist)

        # ---- estimated-time list schedule for the two compute engines ----
        # (order only shapes performance; semaphores enforce correctness)
        NSB = 1 / 360.0         # ns per byte at 360 GB/s
        SEM_DMA, SEM_X = 900.0, 250.0

        def _chunk_bytes(key):
            band = bands[key[0]]
            return len(band["tiles"]) * P * band["chunks"][key[1]][1] * 4

        # DMA emission order (must match the sync block below)
        dma_order = [("a", flat[0])] + [("a", k) for k in hoist]
        dma_order += [("p", flat[0])]
        for k in flat[1:]:
            if k not in hoist_set:
                dma_order.append(("a", k))
            dma_order.append(("p", k))
        arrival = {}
        tdma = 2330.0
        for kind, k in dma_order:
            tdma += _chunk_bytes(k) * NSB
            arrival[(kind, k)] = tdma

        ln_keys = [flat[0]] + hoist + [k for k in flat[1:] if k not in hoist_set]

        def _cols(key):
            band = bands[key[0]]
            return len(band["tiles"]) * band["chunks"][key[1]][1]

        # mandatory sequences
        act_mand = [("ln", k) for k in ln_keys]
        dve_mand = []
        for k in flat:
            dve_mand.append(("d", k))
            dve_mand.append(("stst", k))
        # optional squares
        act_opt = []
        dve_opt = []
        for bi, band in enumerate(bands):
            if band["last"]:
                act_opt += [("sqt", (bi, ci))
                            for ci in range(len(band["chunks"]))]
            else:
                for t in band["tiles"]:
                    if band["sq_engine"][t] == "act":
                        act_opt.append(("sqa", (bi, t)))
                    else:
                        dve_opt.append(("sqv", (bi, t)))

        end_time = {}  # (op, key) -> estimated end

        def _dur(op, key):
            if op == "ln":
                return 57 + _cols(key) / 1.2
            if op == "d":
                return 70 + _cols(key) / 0.96
            if op == "stst":
                return 70 + bands[key[0]]["chunks"][key[1]][1] / 0.96
            if op == "sqv":
                return 70 + bands[key[0]]["w"] / 0.96
            if op == "sqa":
                return 250 + bands[key[0]]["w"] / 1.2
            if op == "sqt":
                bi, ci = key
                return 250 + bands[bi]["chunks"][ci][1] / 1.2
            raise AssertionError(op)

        def _ready(op, key):
            if op == "ln":
                return arrival[("a", key)] + SEM_DMA
            if op == "d":
                t = max(arrival[("p", key)] + SEM_DMA,
                        end_time.get(("ln", key), np.inf) + SEM_X)
                return t
            if op == "stst":
                return end_time.get(("d", key), np.inf) + 190
            if op == "sqv":
                bi = key[0]
                lastc = (bi, len(bands[bi]["chunks"]) - 1)
                return end_time.get(("d", lastc), np.inf) + 190
            if op == "sqa":
                bi, t = key
                lastc = (bi, len(bands[bi]["chunks"]) - 1)
                if t == bands[bi]["b"]:
                    return end_time.get(("stst", lastc), np.inf) + SEM_X
                return end_time.get(("d", lastc), np.inf) + SEM_X
            if op == "sqt":
                bi, ci = key
                return end_time.get(("stst", (bi, ci)), np.inf) + SEM_X
            raise AssertionError(op)

        act_order = []
        dve_order = []
        clocks = {"act": 0.0, "dve": 0.0}
        streams = {"act": (act_mand, act_opt, act_order),
                   "dve": (dve_mand, dve_opt, dve_order)}

        def _candidate(eng):
            mand, opt, _ = streams[eng]
            clock = clocks[eng]
            m_start = np.inf
            if mand:
                m_start = max(clock, _ready(*mand[0]))
            best_opt = None
            for o in opt:
                st = max(clock, _ready(*o))
                if st + _dur(*o) <= m_start and (
                        best_opt is None or st < best_opt[0]):
                    best_opt = (st, o)
            if best_opt is not None:
                return (best_opt[0], "o", best_opt[1])
            if mand:
                return (m_start, "m", mand[0])
            return None

        while any(streams[e][0] or streams[e][1] for e in ("act", "dve")):
            cands = {}
            for e in ("act", "dve"):
                c = _candidate(e)
                if c is not None and np.isfinite(c[0]):
                    cands[e] = c
            if not cands:
                # nothing ready anywhere (shouldn't happen): force ACT mand
                e = "act" if streams["act"][0] else "dve"
                mand, opt, order = streams[e]
                op = mand.pop(0) if mand else opt.pop(0)
                st = max(clocks[e], 0.0)
                end_time[op] = st + _dur(*op)
                clocks[e] = end_time[op]
                order.append(op)
                continue
            e = min(cands, key=lambda x: cands[x][0])
            st, kind, op = cands[e]
            mand, opt, order = streams[e]
            if kind == "m":
                mand.pop(0)
            else:
                opt.remove(op)
            end_time[op] = st + _dur(*op)
            clocks[e] = end_time[op]
            order.append(op)

        @block.sync
        def _(sync):
            def dma_a(key):
                i = chunk_id[key]
                with nc.allow_non_contiguous_dma(
                        reason="degenerate tiny chunk widths"):
                    sync.dma_start(
                        sbuf_chunk(align_sb, *key), dram_chunk(align_d, *key)
                    ).then_inc(s_align[i], 16)

            def dma_p(key):
                i = chunk_id[key]
                with nc.allow_non_contiguous_dma(
                        reason="degenerate tiny chunk widths"):
                    sync.dma_start(
                        sbuf_chunk(pred_sb, *key), dram_chunk(pred_d, *key)
                    ).then_inc(s_pred[i], 16)

            dma_a(flat[0])
            for key in hoist:
                dma_a(key)
            dma_p(flat[0])
            for key in flat[1:]:
                if key not in hoist_set:
                    dma_a(key)
                dma_p(key)
            sync.wait_ge(s_sqa, n_sqa_total)
            if n_sqv_total:
                sync.wait_ge(s_sqv, n_sqv_total)
            sync.dma_start(out_d[:, :], rs_sb[:, :]).then_inc(s_out, 16)
            sync.wait_ge(s_out, 16)

        @block.gpsimd
        def _(gpsimd):
            # lens via the SWDGE queue: keeps the HWDGE ring for bulk traffic
            gpsimd.dma_start(lens_sb[:, :], lens_d[:, :]).then_inc(s_lens, 16)
            # f32 ramp 0..T-1 (exact below 2^24)
            gpsimd.iota(
                iota_f[:, :], pattern=[[1, T]], base=0, channel_multiplier=0,
                allow_small_or_imprecise_dtypes=True,
            ).then_inc(s_iota, 1)

        @block.vector
        def _(vector):
            vector.wait_ge(s_iota, 1)
            vector.wait_ge(s_lens, 16)

            def emit_sq(bi, t):
                band = bands[bi]
                ti = band["tiles"].index(t)
                dsl = d_region(bi)
                rcol = band["rs"][t]
                vector.wait_ge(s_d, band_d_done[bi])  # RAW: band d complete
                vector.scalar_tensor_tensor(
                    out=dsl[:, ti, :],
                    in0=dsl[:, ti, :],
                    scalar=1.0,
                    in1=dsl[:, ti, :],
                    op0=mybir.AluOpType.mult,
                    op1=mybir.AluOpType.mult,
                    accum_out=rs_sb[:, rcol:rcol + 1],
                ).then_inc(s_sqv, 1)

            for op, key in dve_order:
                if op == "sqv":
                    emit_sq(*key)
                    continue
                bi, ci = key
                band = bands[bi]
                i = chunk_id[key]
                o, w = band["chunks"][ci]
                if op == "d":
                    vector.wait_ge(s_pred[i], 16)
                    vector.wait_ge(s_la[i], 1)
                    vector.tensor_sub(
                        d_chunk(bi, ci), sbuf_chunk(pred_sb, bi, ci),
                        sbuf_chunk(align_sb, bi, ci),
                    ).then_inc(s_d, 1)
                else:  # stst: masked diagonal slice (diag = first tile)
                    vector.wait_ge(s_d, d_idx[key])  # same-engine RAW
                    vector.scalar_tensor_tensor(
                        out=dm_chunk(bi, ci),
                        in0=iota_f[:, o:o + w],
                        scalar=lens_sb[:, band["b"]:band["b"] + 1],
                        in1=d_chunk(bi, ci)[:, 0, :],
                        op0=mybir.AluOpType.is_lt,
                        op1=mybir.AluOpType.mult,
                    ).then_inc(s_dm, 1)

        @block.scalar
        def _(scalar):
            n_sq = 0

            def ln(key):
                i = chunk_id[key]
                scalar.wait_ge(s_align[i], 16)
                scalar.activation(
                    sbuf_chunk(align_sb, *key), sbuf_chunk(align_sb, *key),
                    mybir.ActivationFunctionType.Ln,
                ).then_inc(s_la[i], 1)

            def square(src, w, rcol):
                nonlocal n_sq
                if n_sq >= 2:
                    # same-engine WAW on alternating sq_sb scratch
                    scalar.wait_ge(s_sqa, n_sq - 1)
                scalar.activation(
                    sq_sb[:, n_sq % 2, :w], src,
                    mybir.ActivationFunctionType.Square,
                    accum_out=rs_sb[:, rcol:rcol + 1],
                ).then_inc(s_sqa, 1)
                n_sq += 1

            for op, key in act_order:
                if op == "ln":
                    ln(key)
                elif op == "sqa":
                    bi, t = key
                    band = bands[bi]
                    if t == band["b"]:
                        scalar.wait_ge(s_dm, band_dm_done[bi])
                        src = dm_sb[:, dm_off[bi]:dm_off[bi] + band["w"]]
                    else:
                        scalar.wait_ge(s_d, band_d_done[bi])
                        src = d_region(bi)[:, band["tiles"].index(t), :]
                    square(src, band["w"], band["rs"][t])
                else:  # sqt: tail chunk masked square
                    bi, ci = key
                    scalar.wait_ge(s_dm, d_idx[(bi, ci)])
                    square(dm_chunk(bi, ci), bands[bi]["chunks"][ci][1],
                           bands[bi]["rs"][ci])

    return nc, bands, n_rs


def _get_module(W):
    key = tuple(W)
    if key not in _CACHE:
        _CACHE[key] = _build_module(W)
    return _CACHE[key]


def _plan_sharding(lens):
    """Sorted, rank-interleaved sharding. Returns (rows[c] global row ids per
    core in [tile, partition] order, W per-tile max lengths)."""
    order = np.argsort(lens, kind="stable")
    W = []
    for t in range(N_TILES):
        grp = lens[order[t * GROUP:(t + 1) * GROUP]]
        W.append(int(grp.max()))
    rows = []
    for c in range(N_CORES):
        ids = np.empty(RPC, dtype=np.int64)
        for t in range(N_TILES):
            ids[t * P:(t + 1) * P] = order[
                t * GROUP + c + N_CORES * np.arange(P)]
        rows.append(ids)
    return rows, W


def _combine(results, lens, rows, bands):
    total = 0.0
    for c in range(N_CORES):
        rs = np.asarray(results[c]["rowsums"], dtype=np.float64)  # [P, n_rs]
        rows_sum = np.zeros((P, N_TILES))
        for band in bands:
            if band["last"]:
                for ci in range(len(band["chunks"])):
                    rows_sum[:, band["b"]] += rs[:, band["rs"][ci]]
            else:
                for t in band["tiles"]:
                    rows_sum[:, t] += rs[:, band["rs"][t]]
        per_row = rows_sum.T.reshape(RPC)
        lc = lens[rows[c]].astype(np.float64)
        total += np.sum(per_row / lc)
    return np.array(total / B, dtype=np.float32)


def run(inputs, trace: bool = False):
    """Returns (output, BassKernelResults). trace=True also profiles core 0."""
    pred = np.asarray(inputs["pred"], dtype=np.float32)
    align = np.asarray(inputs["alignment"], dtype=np.float32)
    lens = np.asarray(inputs["token_lengths"])

    rows, W = _plan_sharding(lens)
    nc, bands, n_rs = _get_module(W)

    in_maps = []
    for c in range(N_CORES):
        ids = rows[c]
        lens_c = lens[ids].astype(np.float32)
        in_maps.append({
            "pred": np.ascontiguousarray(pred[ids]),
            "align": np.ascontiguousarray(align[ids]),
            "lens": np.ascontiguousarray(lens_c.reshape(N_TILES, P).T),
        })

    res = run_bass_kernel_spmd(nc, in_maps, core_ids=list(range(N_CORES)), trace=trace)
    return _combine(res.results, lens, rows, bands), res


def kernel(**inputs) -> np.ndarray:
    out, _ = run(inputs, trace=False)
    return out
